# revision 17
# baseline (speedup 1.0000x reference)
"""GATv2 + GraphNorm block on 8 trn2 NeuronCores.

Strategy (graph/data parallel per sharding hint):
- Nodes are partitioned by destination range across the 8 cores
  (6250 nodes each). Each core handles the incoming edges (messages)
  of its destination nodes; weights are replicated.
- Host builds, per core, a degree-sorted padded "grid" of messages:
  destinations are sorted by in-degree and packed into blocks of 128
  (the partition dim); each block is padded to the max degree in its
  group. Source-node features for every slot are laid out transposed
  [feature, slot] so the device streams them contiguously.
- Device pipeline (per slot chunk): W_l matmul (PE) -> +x_r broadcast
  (DVE) -> LeakyReLU (ACT) -> replicated attention matmul (PE) ->
  exp (ACT) -> ex*z (DVE) -> segment sums for numerator/denominator
  via identity-matmul PSUM-accumulation folds (PE).
- Softmax denominators / numerators combine per block; GraphNorm uses
  per-core partial sums combined on host (the only cross-core data is
  2x128 floats per core), then a second tiny device pass applies the
  per-feature affine normalization.
"""

import numpy as np

N = 50000
F = 128
H = 4
C = 32
NEG_SLOPE = 0.2
EPS = 1e-5
NCORES = 8
NLOC = N // NCORES  # 6250
P = 128
NBLK = (NLOC + P - 1) // P  # 49
NLOCP = NBLK * P  # 6272 padded local dst count
PAD_BIG = 1.0e12
SLOT_CAP = 4096  # max grid columns per group (per-partition bytes stay sane)
NB_CAP = 8  # max blocks per group (PSUM fold region = nb*128 <= 1024)

_cache = {}


def _plan_groups(dmax_per_block):
    """Common (nb, D) schedule for all cores from per-block max degrees.

    Blocks are in descending max-degree order, so a group's D is its first
    block's. Caps: nb*D*128 columns <= SLOT_CAP, nb <= NB_CAP.
    """
    groups = []
    b = 0
    while b < NBLK:
        D = max(int(dmax_per_block[b]), 1)
        nb = 1
        while (
            b + nb < NBLK
            and nb < NB_CAP
            and (nb + 1) * D * P <= SLOT_CAP
        ):
            nb += 1
        groups.append((nb, D))
        b += nb
    return groups


def _build_device_programs(groups):
    import concourse.bacc as bacc
    import concourse.bass as bass
    import concourse.mybir as mybir
    import concourse.tile as tile

    S_total = sum(nb * D * P for nb, D in groups)

    nc = bacc.Bacc(None, target_bir_lowering=False)
    dt = mybir.dt.float32
    xgT = nc.dram_tensor("xgT", [P, S_total], dt, kind="ExternalInput")
    xdT = nc.dram_tensor("xdT", [P, NLOCP], dt, kind="ExternalInput")
    wl = nc.dram_tensor("wl", [P, P], dt, kind="ExternalInput")
    wr = nc.dram_tensor("wr", [P, P], dt, kind="ExternalInput")
    a2r = nc.dram_tensor("a2r", [P, P], dt, kind="ExternalInput")
    ident = nc.dram_tensor("ident", [P, P], dt, kind="ExternalInput")
    bias_v = nc.dram_tensor("bias_v", [P, 1], dt, kind="ExternalInput")
    outT = nc.dram_tensor("outT", [P, NLOCP], dt, kind="ExternalOutput")

    G = len(groups)
    with tile.TileContext(nc) as tc:
        with (
            tc.tile_pool(name="const", bufs=1) as cp,
            tc.tile_pool(name="gxp", bufs=2) as gxp,
            tc.tile_pool(name="stream", bufs=1) as sp,
            tc.tile_pool(name="xdp", bufs=2) as xdp,
            tc.tile_pool(name="ps", bufs=2, space="PSUM") as pp,
            tc.tile_pool(name="psf", bufs=1, space="PSUM") as ppf,
            tc.tile_pool(name="small", bufs=2) as smp,
        ):
            wl_t = cp.tile([P, P], dt)
            nc.sync.dma_start(wl_t[:], wl[:])
            wr_t = cp.tile([P, P], dt)
            nc.sync.dma_start(wr_t[:], wr[:])
            a2r_t = cp.tile([P, P], dt)
            nc.sync.dma_start(a2r_t[:], a2r[:])
            id_t = cp.tile([P, P], dt)
            nc.sync.dma_start(id_t[:], ident[:])
            bias_t = cp.tile([P, 1], dt)
            nc.sync.dma_start(bias_t[:], bias_v[:])
            xr_t = cp.tile([P, NLOCP], dt)
            out_t = cp.tile([P, NLOCP], dt)

            # x_r = W_r^T @ xdT  (per 512-wide chunk)
            for j in range(0, NLOCP, 512):
                w = min(512, NLOCP - j)
                xd_t = xdp.tile([P, 512], dt, tag="xd")
                nc.sync.dma_start(xd_t[:, :w], xdT[:, j : j + w])
                xr_ps = pp.tile([P, 512], dt, tag="xlps")
                nc.tensor.matmul(
                    out=xr_ps[:, :w], lhsT=wr_t[:], rhs=xd_t[:, :w],
                    start=True, stop=True,
                )
                nc.scalar.copy(out=xr_t[:, j : j + w], in_=xr_ps[:, :w])

            off = 0
            for g, (nb, D) in enumerate(groups):
                S = nb * D * P
                gx = gxp.tile([P, S], dt, tag="gx")
                nc.sync.dma_start(gx[:], xgT[:, off : off + S])
                z_t = sp.tile([P, S], dt, tag="z")
                lr_t = sp.tile([P, S], dt, tag="lr")
                ex_t = sp.tile([P, S], dt, tag="ex")

                # chunks of up to 4 d-slices (512 cols) within one block
                chunks = []
                for b in range(nb):
                    d0 = 0
                    while d0 < D:
                        dd = min(4, D - d0)
                        chunks.append((b, d0, dd))
                        d0 += dd

                # z = W_l^T @ gx + x_r (broadcast over d)
                for (b, d0, dd) in chunks:
                    col = b * D * P + d0 * P
                    w = dd * P
                    xl_ps = pp.tile([P, 512], dt, tag="xlps")
                    nc.tensor.matmul(
                        out=xl_ps[:, :w], lhsT=wl_t[:],
                        rhs=gx[:, col : col + w], start=True, stop=True,
                    )
                    xr_b = (
                        xr_t[:, (g_blk0(groups, g) + b) * P : (g_blk0(groups, g) + b + 1) * P]
                        .unsqueeze(1)
                        .to_broadcast([P, dd, P])
                    )
                    nc.vector.tensor_tensor(
                        out=z_t[:, col : col + w].rearrange("p (d q) -> p d q", q=P),
                        in0=xl_ps[:, :w].rearrange("p (d q) -> p d q", q=P),
                        in1=xr_b,
                        op=mybir.AluOpType.add,
                    )

                # leaky relu over the whole group: max(0.2*z, z) on DVE
                # (exact; ACT Lrelu LUT is unverified here and CoreSim lacks it)
                nc.vector.scalar_tensor_tensor(
                    out=lr_t[:],
                    in0=z_t[:],
                    scalar=NEG_SLOPE,
                    in1=z_t[:],
                    op0=mybir.AluOpType.mult,
                    op1=mybir.AluOpType.max,
                )

                # score_rep = A2R^T @ lrelu ; ex = exp(score_rep)
                for (b, d0, dd) in chunks:
                    col = b * D * P + d0 * P
                    w = dd * P
                    sc_ps = pp.tile([P, 512], dt, tag="scps")
                    nc.tensor.matmul(
                        out=sc_ps[:, :w], lhsT=a2r_t[:],
                        rhs=lr_t[:, col : col + w], start=True, stop=True,
                    )
                    nc.scalar.activation(
                        out=ex_t[:, col : col + w], in_=sc_ps[:, :w],
                        func=mybir.ActivationFunctionType.Exp,
                    )

                # m = ex * z
                m_t = sp.tile([P, S], dt, tag="m")
                nc.vector.tensor_tensor(
                    out=m_t[:], in0=ex_t[:], in1=z_t[:], op=mybir.AluOpType.mult
                )

                # segment folds: agg[p, b*128+q] = sum_d m[p, (b,d,q)]
                # via per-d matmul accumulation into one PSUM region
                agg_ps = ppf.tile([P, nb * P], dt, tag="aggps")
                den_ps = ppf.tile([P, nb * P], dt, tag="denps")
                for b in range(nb):
                    for d in range(D):
                        col = b * D * P + d * P
                        nc.tensor.matmul(
                            out=agg_ps[:, b * P : (b + 1) * P],
                            lhsT=id_t[:],
                            rhs=m_t[:, col : col + P],
                            start=(d == 0), stop=(d == D - 1),
                        )
                for b in range(nb):
                    for d in range(D):
                        col = b * D * P + d * P
                        nc.tensor.matmul(
                            out=den_ps[:, b * P : (b + 1) * P],
                            lhsT=id_t[:],
                            rhs=ex_t[:, col : col + P],
                            start=(d == 0), stop=(d == D - 1),
                        )

                # out = agg / den - x_r + bias
                b0 = g_blk0(groups, g)
                r_t = smp.tile([P, nb * P], dt, tag="recip")
                nc.vector.tensor_scalar_add(r_t[:], den_ps[:], 1e-30)
                nc.vector.reciprocal(r_t[:], r_t[:])
                t_t = smp.tile([P, nb * P], dt, tag="tt")
                nc.vector.tensor_tensor(
                    out=t_t[:], in0=agg_ps[:], in1=r_t[:], op=mybir.AluOpType.mult
                )
                nc.vector.scalar_tensor_tensor(
                    out=out_t[:, b0 * P : (b0 + nb) * P],
                    in0=t_t[:],
                    scalar=bias_t[:, :1],
                    in1=xr_t[:, b0 * P : (b0 + nb) * P],
                    op0=mybir.AluOpType.add,
                    op1=mybir.AluOpType.subtract,
                )

                off += S

            nc.sync.dma_start(outT[:], out_t[:])
    nc.compile()
    return nc, S_total


def g_blk0(groups, g):
    return sum(nb for nb, _ in groups[:g])


def _prep(x, edge_index, W_l, W_r, att, bias):
    """Host-side sharding/preprocessing. Returns per-core in_maps + metadata."""
    x = np.asarray(x, dtype=np.float32)
    ei = np.asarray(edge_index)
    W_l = np.asarray(W_l, dtype=np.float32)
    W_r = np.asarray(W_r, dtype=np.float32)
    att = np.asarray(att, dtype=np.float32)
    bias = np.asarray(bias, dtype=np.float32)

    n = x.shape[0]
    ar = np.arange(n, dtype=np.int64)
    src_all = np.concatenate([ei[0].astype(np.int64), ar])
    dst_all = np.concatenate([ei[1].astype(np.int64), ar])

    # magic pad row: pad-slot scores land in [-85, -25] for every head
    # (inside the ACT Exp LUT range; exp(score) <= 1e-11 => no contribution)
    att_flat = att.reshape(-1)
    svec = np.where(att_flat >= 0.0, 1.0, -1.0).astype(np.float64)
    g = np.array(
        [
            np.sum(np.abs(att[h]) * np.where(att[h] >= 0, NEG_SLOPE, 1.0))
            for h in range(H)
        ]
    )
    B = 80.0 / g.max()
    xl_target = (-B) * svec
    v_pad = np.linalg.solve(W_l.astype(np.float64).T, xl_target).astype(np.float32)
    x_aug = np.vstack([x, v_pad[None, :]])  # row N = pad

    cores = []
    deg_sorted_all = []
    for c in range(NCORES):
        lo, hi = c * NLOC, (c + 1) * NLOC
        m = (dst_all >= lo) & (dst_all < hi)
        es = src_all[m]
        ed = (dst_all[m] - lo).astype(np.int64)
        deg = np.bincount(ed, minlength=NLOC)
        order = np.argsort(-deg, kind="stable")
        deg_s = deg[order]
        cores.append((es, ed, deg, order))
        deg_sorted_all.append(deg_s)

    # common block max-degree schedule across cores
    dmax_blk = np.zeros(NBLK, dtype=np.int64)
    for c in range(NCORES):
        ds = deg_sorted_all[c]
        for b in range(NBLK):
            seg = ds[b * P : (b + 1) * P]
            if len(seg):
                dmax_blk[b] = max(dmax_blk[b], int(seg.max()) if len(seg) else 0)
    dmax_blk = np.maximum(dmax_blk, 1)
    groups = _plan_groups(dmax_blk)

    # per-group D and block offsets
    blkD = np.zeros(NBLK, dtype=np.int64)
    blk_group = np.zeros(NBLK, dtype=np.int64)
    col0_blk = np.zeros(NBLK, dtype=np.int64)
    off = 0
    b = 0
    for gi, (nb, D) in enumerate(groups):
        for k in range(nb):
            blkD[b] = D
            blk_group[b] = gi
            col0_blk[b] = off + k * D * P
            b += 1
        off += nb * D * P
    S_total = off

    in_maps = []
    metas = []
    for c in range(NCORES):
        es, ed, deg, order = cores[c]
        pos = np.empty(NLOC, dtype=np.int64)
        pos[order] = np.arange(NLOC)
        # rank of each edge within its destination
        perm = np.argsort(ed, kind="stable")
        ed_s = ed[perm]
        es_s = es[perm]
        uniq, start = np.unique(ed_s, return_index=True)
        counts = np.diff(np.r_[start, len(ed_s)])
        ranks = np.arange(len(ed_s)) - np.repeat(start, counts)
        pb = pos[ed_s]  # position of dst in sorted order
        blk = pb // P
        q = pb % P
        cols = col0_blk[blk] + ranks * P + q
        col_src = np.full(S_total, n, dtype=np.int64)  # pad row id
        col_src[cols] = es_s
        xg = x_aug[col_src]  # [S_total, 128]
        xgT = np.ascontiguousarray(xg.T)

        gd = np.zeros(NLOCP, dtype=np.int64)
        gd[: NLOC] = order + c * NLOC
        xd = np.zeros((NLOCP, F), dtype=np.float32)
        xd[:NLOC] = x[gd[:NLOC]]
        xdT = np.ascontiguousarray(xd.T)

        a2r = np.zeros((P, P), dtype=np.float32)
        for h in range(H):
            a2r[h * C : (h + 1) * C, h * C : (h + 1) * C] = np.tile(
                att[h][:, None], (1, C)
            )

        in_maps.append(
            {
                "xgT": xgT,
                "xdT": xdT,
                "wl": W_l,
                "wr": W_r,
                "a2r": a2r,
                "ident": np.eye(P, dtype=np.float32),
                "bias_v": bias.reshape(P, 1),
            }
        )
        metas.append(order)
    return in_maps, metas, groups, S_total


def _run_sim(nc, in_maps):
    """CoreSim fallback (GAT_SIM=1): simulate each core on host."""
    from concourse.bass_interp import CoreSim

    class R:
        results = []

    for m in in_maps:
        sim = CoreSim(nc, trace=False)
        for k, v in m.items():
            sim.tensor(k)[:] = v
        sim.simulate()
        R.results.append({"outT": np.array(sim.tensor("outT"))})
    return R


def kernel(x, edge_index, W_l, W_r, att, bias, gn_weight, gn_bias, gn_mean_scale):
    import os

    from concourse.bass_utils import run_bass_kernel_spmd

    in_maps, metas, groups, S_total = _prep(x, edge_index, W_l, W_r, att, bias)

    key = ("p1", tuple(groups))
    if key not in _cache:
        _cache[key] = _build_device_programs(groups)
    nc, S_chk = _cache[key]
    assert S_chk == S_total

    if os.environ.get("GAT_SIM") == "1":
        res = _run_sim(nc, in_maps)
    else:
        res = run_bass_kernel_spmd(nc, in_maps, core_ids=list(range(NCORES)))

    gn_weight = np.asarray(gn_weight, dtype=np.float32)
    gn_bias = np.asarray(gn_bias, dtype=np.float32)
    gn_mean_scale = np.asarray(gn_mean_scale, dtype=np.float32)

    ssum = np.zeros(F, dtype=np.float64)
    ssq = np.zeros(F, dtype=np.float64)
    outs = []
    for c in range(NCORES):
        y = res.results[c]["outT"].T[:NLOC].astype(np.float64)  # real rows only
        ssum += y.sum(axis=0)
        ssq += (y * y).sum(axis=0)
        outs.append(res.results[c]["outT"])

    n = x.shape[0]
    mean = ssum / n
    # var of (y - s*mean): E[y^2] - 2 s mean E[y] + s^2 mean^2
    s = gn_mean_scale.astype(np.float64)
    ey2 = ssq / n
    ey = ssum / n
    var = ey2 - 2 * s * mean * ey + (s * mean) ** 2
    A = (gn_weight.astype(np.float64) / np.sqrt(var + EPS)).astype(np.float32)
    B = (gn_bias.astype(np.float64) - A * s * mean).astype(np.float32)

    out = np.empty((n, F), dtype=np.float32)
    for c in range(NCORES):
        yT = outs[c]  # [128, NLOCP]
        y = yT.T[:NLOC]  # sorted-order rows
        y = y * A[None, :] + B[None, :]
        order = metas[c]
        out[order + c * NLOC] = y
    return out
